# revision 9
# baseline (speedup 1.0000x reference)
"""Trainium2 Bass kernel for DifferentiableShockProximity.

Math: is_shock at interface k (k=1..nx-1) reduces to state[k] > state[k-1]
(Greenshields Lax condition collapses to density increase). The reference's
O(nx^2) masked-distance min is a 1D nearest-shock distance transform:

    min_dist(i) = dx * min( (i+0.5) + min_{k<=i}(u_k - k),
                           -(i+0.5) + min_{k>i}(u_k + k) )

with u_k = 0 at shocks, BIG elsewhere. Prefix/suffix mins run as hardware
tensor_tensor_scan ops along the free axis in a [128 partitions = (row,
chunk), 128 free = position-in-chunk] layout, with a tiny cross-chunk
combine through a PE transpose + segmented scan over chunk totals.

All index arithmetic is exact in f32: values live in [-(2^21+2^11), 2^22],
integers (+0.5 offsets) below 2^24. "0 means +inf" encoding: every real
scan value is shifted by -2^21 so it is negative; the multiplicative
segment-reset of the cross-chunk scan then yields 0, a natural +inf.

Data parallel over batch: 64 rows -> 8 cores x 8 rows. Host pads each
row-chunk with its left neighbor element so the shifted compare needs no
cross-partition traffic: input per core is [128, 129].
"""

import os
import sys

import numpy as np

for _p in (
    "/root/.axon_site/_ro/trn_rl_repo",
    "/opt/trn_rl_repo",
):
    if os.path.isdir(_p) and _p not in sys.path:
        sys.path.append(_p)

import concourse.bass as bass
import concourse.mybir as mybir
from concourse import bacc, masks
from concourse.bass_utils import run_bass_kernel_spmd
from concourse.tile import TileContext

N_CORES = 8
B, NX = 64, 2048
R = B // N_CORES  # rows per core
CCH = 16          # chunks per row
F = 128           # chunk length
P = R * CCH       # 128 partitions
C_OFS = float(2 ** 21)   # shift making every scan value negative
U_BIG = float(2 ** 20)   # "no shock" marker (index units)
SIGMA = 0.05

FP = mybir.dt.float32
Alu = mybir.AluOpType


def build_nc(compile: bool = True) -> bass.Bass:
    nc = bacc.Bacc(
        "TRN2", target_bir_lowering=False, debug=False, num_devices=N_CORES
    )
    sp = nc.declare_dram_parameter("sp", [P, F + 1], FP, isOutput=False)
    mrow = nc.declare_dram_parameter("mrow", [2, P], FP, isOutput=False)
    cols = nc.declare_dram_parameter("cols", [P, 5], FP, isOutput=False)
    out = nc.declare_dram_parameter("out", [P, F], FP, isOutput=True)

    with TileContext(nc) as tc:
        with (
            tc.tile_pool(name="main", bufs=1) as pool,
            tc.tile_pool(name="ps", bufs=1, space="PSUM") as pps,
        ):
            sp_t = pool.tile([P, F + 1], FP)
            nc.sync.dma_start(out=sp_t[:], in_=sp[:])
            col_t = pool.tile([P, 5], FP)
            nc.sync.dma_start(out=col_t[:], in_=cols[:])
            # engine ops must start at partition 0, so the two segment-mask
            # rows live in separate [1, P] tiles
            mr0 = pool.tile([1, P], FP)
            nc.sync.dma_start(out=mr0[:], in_=mrow[0:1, :])
            mr1 = pool.tile([1, P], FP)
            nc.sync.dma_start(out=mr1[:], in_=mrow[1:2, :])
            kbn, kbp, kb2, kb3, dxs = (col_t[:, i : i + 1] for i in range(5))

            idp = pool.tile([P, P], FP)
            masks.make_identity(nc, idp[:])
            ones1 = pool.tile([1, 1], FP)
            nc.gpsimd.memset(ones1[:], 1.0)

            iot = pool.tile([P, F], FP)
            nc.gpsimd.iota(
                iot[:], pattern=[[1, F]], base=0, channel_multiplier=0,
                allow_small_or_imprecise_dtypes=True,
            )

            # mask: shock at interface k = chunk*128+f  <=>  s[k] > s[k-1]
            mask = pool.tile([P, F], FP)
            nc.vector.tensor_tensor(
                mask[:], sp_t[:, 1 : F + 1], sp_t[:, 0:F], Alu.is_gt
            )
            # u = BIG*(1-mask); the left pad element (2.0) kills k=0.
            u = pool.tile([P, F], FP)
            nc.scalar.activation(
                u[:], mask[:], mybir.ActivationFunctionType.Copy,
                bias=U_BIG, scale=-U_BIG,
            )

            # STT-class ops (scalar_tensor_tensor / tensor_tensor_scan /
            # tensor_scalar-with-AP) have few sync-wait slots in the ISA;
            # warm their cross-engine deps on the DVE clock with cheap
            # plain tensor_tensor ops so they need at most one fresh wait.
            scr = pool.tile([P, 1], FP)
            nc.vector.tensor_tensor(scr[:], u[:, 0:1], iot[:, 0:1], Alu.max)
            scr2 = pool.tile([P, 1], FP)
            nc.vector.tensor_tensor(scr2[:], scr[:], col_t[:, 0:1], Alu.max)

            # vt = u - k - C, wt = u + k - C   (k = kb[p] + f, all < 0)
            vt = pool.tile([P, F], FP)
            nc.vector.scalar_tensor_tensor(
                vt[:], u[:], kbn, iot[:], Alu.add, Alu.subtract
            )
            wt = pool.tile([P, F], FP)
            nc.vector.scalar_tensor_tensor(
                wt[:], u[:], kbp, iot[:], Alu.add, Alu.add
            )

            # chunk-local inclusive prefix-min of vt
            pf = pool.tile([P, F], FP)
            nc.vector.tensor_tensor_scan(
                pf[:], vt[:], vt[:], 0.0, Alu.min, Alu.min
            )
            # chunk-local exclusive suffix-min of wt (shift left, then
            # reversed scan); trailing slot = 0 (= +inf)
            wsh = pool.tile([P, F], FP)
            nc.vector.tensor_copy(wsh[:, 0 : F - 1], wt[:, 1:F])
            nc.gpsimd.memset(wsh[:, F - 1 : F], 0.0)
            wx = pool.tile([P, F], FP)
            nc.vector.tensor_tensor_scan(
                wx[:, ::-1], wsh[:, ::-1], wsh[:, ::-1], 0.0, Alu.min, Alu.min
            )

            # chunk totals -> [P, 2] column pair
            tt = pool.tile([P, 2], FP)
            nc.vector.tensor_copy(tt[:, 0:1], pf[:, F - 1 : F])
            nc.vector.tensor_tensor(tt[:, 1:2], wt[:, 0:1], wx[:, 0:1], Alu.min)

            # PE transpose each totals column to a [1, P] row
            tp0 = pps.tile([1, P], FP)
            nc.tensor.transpose(tp0[:], tt[:, 0:1], idp[:])
            tp1 = pps.tile([1, P], FP)
            nc.tensor.transpose(tp1[:], tt[:, 1:2], idp[:])
            ts0 = pool.tile([1, P], FP)
            nc.scalar.copy(ts0[:], tp0[:])
            ts1 = pool.tile([1, P], FP)
            nc.scalar.copy(ts1[:], tp1[:])

            # staged rows for segmented exclusive scans over chunk index
            st0 = pool.tile([1, P], FP)
            nc.vector.tensor_copy(st0[0:1, 1:P], ts0[0:1, 0 : P - 1])
            nc.gpsimd.memset(st0[0:1, 0:P:CCH], 0.0)
            st1 = pool.tile([1, P], FP)
            nc.vector.tensor_copy(st1[0:1, 0 : P - 1], ts1[0:1, 1:P])
            nc.gpsimd.memset(st1[0:1, CCH - 1 : P : CCH], 0.0)

            scr3 = pool.tile([1, 1], FP)
            nc.vector.tensor_tensor(
                scr3[:], mr0[0:1, 0:1], st0[0:1, 0:1], Alu.max
            )
            scr4 = pool.tile([1, 1], FP)
            nc.vector.tensor_tensor(
                scr4[:], mr1[0:1, 0:1], st1[0:1, CCH - 1 : CCH], Alu.max
            )

            e20 = pool.tile([1, P], FP)
            nc.vector.tensor_tensor_scan(
                e20[:], mr0[:], st0[:], 0.0, Alu.mult, Alu.min
            )
            e21 = pool.tile([1, P], FP)
            nc.vector.tensor_tensor_scan(
                e21[0:1, ::-1], mr1[0:1, ::-1], st1[0:1, ::-1], 0.0,
                Alu.mult, Alu.min,
            )

            # back to per-partition columns
            ep0 = pps.tile([P, 1], FP)
            nc.tensor.transpose(ep0[:], e20[:], ones1[:])
            ep1 = pps.tile([P, 1], FP)
            nc.tensor.transpose(ep1[:], e21[:], ones1[:])

            # md_f = min(pf, E0) + (kb + C + 0.5) + f
            # md_b = min(wx, E1) + (C - kb - 0.5) - f
            mdf = pool.tile([P, F], FP)
            nc.vector.tensor_scalar(
                mdf[:], pf[:], ep0[:, 0:1], kb2, Alu.min, Alu.add
            )
            mdb = pool.tile([P, F], FP)
            nc.vector.tensor_scalar(
                mdb[:], wx[:], ep1[:, 0:1], kb3, Alu.min, Alu.add
            )
            xf = pool.tile([P, F], FP)
            nc.vector.tensor_add(xf[:], mdf[:], iot[:])
            yb = pool.tile([P, F], FP)
            nc.vector.tensor_sub(yb[:], mdb[:], iot[:])
            md = pool.tile([P, F], FP)
            nc.vector.tensor_tensor(md[:], xf[:], yb[:], Alu.min)

            # out = exp(md * (-dx/sigma))
            ot = pool.tile([P, F], FP)
            nc.scalar.activation(
                ot[:], md[:], mybir.ActivationFunctionType.Exp, scale=dxs
            )
            nc.sync.dma_start(out=out[:], in_=ot[:])
    if compile:
        nc.compile()
    return nc


_NC_CACHE: bass.Bass | None = None


def _get_nc() -> bass.Bass:
    global _NC_CACHE
    if _NC_CACHE is None:
        _NC_CACHE = build_nc()
    return _NC_CACHE


def _host_inputs(state: np.ndarray, dx: float) -> list[dict[str, np.ndarray]]:
    s = np.ascontiguousarray(
        np.asarray(state, dtype=np.float32).reshape(B, NX)
    )
    # per-core [P, F+1]: partition (r, c) holds s[row, c*128-1 : c*128+128]
    # with a 2.0 pad for the non-existent s[row, -1] (kills interface k=0).
    padded = np.concatenate(
        [np.full((B, 1), 2.0, np.float32), s], axis=1
    )  # [B, NX+1]
    cidx = np.arange(CCH)[:, None] * F + np.arange(F + 1)[None, :]  # [16,129]

    p_idx = np.arange(P)
    kb = (p_idx % CCH).astype(np.float32) * F
    cols = np.stack(
        [
            -kb - C_OFS,
            kb - C_OFS,
            kb + C_OFS + 0.5,
            C_OFS - kb - 0.5,
            np.full(P, -float(dx) / SIGMA, np.float32),
        ],
        axis=1,
    ).astype(np.float32)

    j = np.arange(P)
    mrow = np.ones((2, P), np.float32)
    mrow[0, j % CCH == 0] = 0.0
    mrow[1, j % CCH == CCH - 1] = 0.0

    in_maps = []
    for core in range(N_CORES):
        rows = padded[core * R : (core + 1) * R]  # [R, NX+1]
        sp = rows[:, cidx.ravel()].reshape(R * CCH, F + 1)
        in_maps.append(
            {
                "sp": np.ascontiguousarray(sp),
                "mrow": mrow,
                "cols": cols,
            }
        )
    return in_maps


def kernel(state: np.ndarray, dx) -> np.ndarray:
    dxv = float(np.asarray(dx).reshape(()))
    in_maps = _host_inputs(state, dxv)
    nc = _get_nc()
    res = run_bass_kernel_spmd(nc, in_maps, list(range(N_CORES))).results
    outs = [res[c]["out"].reshape(R, NX) for c in range(N_CORES)]
    full = np.concatenate(outs, axis=0).astype(np.float32)  # [B, NX]
    return full[:, None, :]


# revision 13
# speedup vs baseline: 1.0569x; 1.0569x over previous
"""Trainium2 Bass kernel for DifferentiableShockProximity.

Math: is_shock at interface k (k=1..nx-1) reduces to state[k] > state[k-1]
(the Greenshields Lax condition collapses to "density increases"). The
reference's O(nx^2) masked-distance min is a 1D nearest-shock distance
transform:

    min_dist(i) = dx * min( (i+0.5) + min_{k<=i}(u_k - k),
                           -(i+0.5) + min_{k>i}(u_k + k) )

with u_k = 0 at shocks, BIG elsewhere. Prefix/suffix mins run as hardware
tensor_tensor_scan ops along the free axis in a [128 partitions = (row,
chunk), 128 free = position-in-chunk] layout, with a small cross-chunk
combine through PE transposes + segmented scans over chunk totals.

All index arithmetic is exact in f32: integers (+0.5 offsets) below 2^24.
"0 means +inf" encoding: every real scan value is shifted by -2^21 so it
is negative; the multiplicative segment-reset of the cross-chunk scan
then yields 0, a natural +inf.

Data parallel over batch: 64 rows -> 8 cores x 8 rows. Host pads each
row-chunk with its left neighbor element so the shifted compare needs no
cross-partition traffic.
"""

import os
import sys

import numpy as np

for _p in (
    "/root/.axon_site/_ro/trn_rl_repo",
    "/opt/trn_rl_repo",
):
    if os.path.isdir(_p) and _p not in sys.path:
        sys.path.append(_p)

import concourse.bass as bass
import concourse.mybir as mybir
from concourse import bacc
from concourse.bass_utils import run_bass_kernel_spmd
from concourse.tile import TileContext
from concourse.vector_clock import ScopedClock

N_CORES = 8
B, NX = 64, 2048
R = B // N_CORES  # rows per core
CCH = 16          # chunks per row
F = 128           # chunk length
P = R * CCH       # 128 partitions
C_OFS = float(2 ** 21)   # shift making every scan value negative
U_BIG = float(2 ** 20)   # "no shock" marker (index units)
SIGMA = 0.05

FP = mybir.dt.float32
Alu = mybir.AluOpType

# host-const layout along the free axis of the single input tensor:
# [0:129)    sp     per-chunk state with left-overlap element
# [129:257)  X1     BIG - C - k        (k = 128*(p%16) + f)
# [257:385)  X2     BIG - C + k
# [385:513)  Z1     k + C + 0.5
# [513:641)  Z2     C - k - 0.5
# [641:769)  idp    identity 128x128
# [769:770)  dxs    -dx/SIGMA
O_SP, O_X1, O_X2, O_Z1, O_Z2, O_ID, O_DXS = 0, 129, 257, 385, 513, 641, 769
W_IN = 770
# second small input: segment masks for the cross-chunk scans
# mrow[0, j] = 0 if j % 16 == 0 else 1 ; mrow[1, j] = 0 if j % 16 == 15 else 1


class _FastTileContext(TileContext):
    """TileContext with a cheap kernel tail.

    The stock exit emits drain + EVSEM-butterfly barrier + sem clear +
    second butterfly (~9 us on HW). This kernel is straight-line: once the
    final sync.drain has waited on the global vector clock, every
    semaphore increment has already happened, so a single sequencer-level
    (sem-only) barrier before the clear is enough, and nothing runs after
    the clear within this execution.
    """

    def _drain_and_barrier(self, tick_clock, wait_clock):
        drain_inst = self.nc.sync.drain()
        wait_clock.add_sem_waits(
            drain_inst.ins, ScopedClock({None: tick_clock.global_clock})
        )
        self.nc.all_engine_barrier(sem_only=True)
        assert self.sems is not None
        popped = self.nc._tile_sem_poison_stack.pop()
        assert popped is self._sem_poison
        self.nc.clear_and_free_semaphores(list(self.sems.allocated().values()))


def build_nc(compile: bool = True) -> bass.Bass:
    nc = bacc.Bacc(
        "TRN2", target_bir_lowering=False, debug=False, num_devices=N_CORES
    )
    inp = nc.declare_dram_parameter("inp", [P, W_IN], FP, isOutput=False)
    mrow = nc.declare_dram_parameter("mrow", [2, P], FP, isOutput=False)
    out = nc.declare_dram_parameter("out", [P, F], FP, isOutput=True)

    with _FastTileContext(nc) as tc:
        with (
            tc.tile_pool(name="main", bufs=1) as pool,
            tc.tile_pool(name="ps", bufs=1, space="PSUM") as pps,
        ):
            it = pool.tile([P, W_IN], FP)
            nc.sync.dma_start(out=it[:], in_=inp[:])
            sp_t = it[:, O_SP : O_SP + F + 1]
            x1 = it[:, O_X1 : O_X1 + F]
            x2 = it[:, O_X2 : O_X2 + F]
            z1 = it[:, O_Z1 : O_Z1 + F]
            z2 = it[:, O_Z2 : O_Z2 + F]
            idp = it[:, O_ID : O_ID + P]
            dxs = it[:, O_DXS : O_DXS + 1]

            mr0 = pool.tile([1, P], FP)
            nc.scalar.dma_start(out=mr0[:], in_=mrow[0:1, :])
            mr1 = pool.tile([1, P], FP)
            nc.scalar.dma_start(out=mr1[:], in_=mrow[1:2, :])
            ones1 = pool.tile([1, 1], FP)
            nc.gpsimd.memset(ones1[:], 1.0)

            # mask: shock at interface k = chunk*128+f  <=>  s[k] > s[k-1]
            mask = pool.tile([P, F], FP)
            nc.vector.tensor_tensor(
                mask[:], sp_t[:, 1 : F + 1], sp_t[:, 0:F], Alu.is_gt
            )
            # vt = u - k - C = mask*(-BIG) + X1, wt = u + k - C = mask*(-BIG) + X2
            vt = pool.tile([P, F], FP)
            nc.vector.scalar_tensor_tensor(
                vt[:], mask[:], -U_BIG, x1, Alu.mult, Alu.add
            )
            wt = pool.tile([P, F], FP)
            nc.vector.scalar_tensor_tensor(
                wt[:], mask[:], -U_BIG, x2, Alu.mult, Alu.add
            )

            # chunk totals via direct reduces (decoupled from the scans)
            tt = pool.tile([P, 2], FP)
            nc.vector.tensor_reduce(
                tt[:, 0:1], vt[:], mybir.AxisListType.X, Alu.min
            )
            nc.vector.tensor_reduce(
                tt[:, 1:2], wt[:], mybir.AxisListType.X, Alu.min
            )

            # chunk-local inclusive prefix-min of vt
            pf = pool.tile([P, F], FP)
            nc.vector.tensor_tensor_scan(
                pf[:], vt[:], vt[:], 0.0, Alu.min, Alu.min
            )
            # chunk-local exclusive suffix-min of wt (shift left, then
            # reversed scan); trailing slot = 0 (= +inf)
            wsh = pool.tile([P, F], FP)
            nc.gpsimd.tensor_copy(wsh[:, 0 : F - 1], wt[:, 1:F])
            nc.gpsimd.memset(wsh[:, F - 1 : F], 0.0)
            wx = pool.tile([P, F], FP)
            nc.vector.tensor_tensor_scan(
                wx[:, ::-1], wsh[:, ::-1], wsh[:, ::-1], 0.0, Alu.min, Alu.min
            )

            # PE transpose each totals column to a [1, P] row
            tp0 = pps.tile([1, P], FP)
            nc.tensor.transpose(tp0[:], tt[:, 0:1], idp)
            tp1 = pps.tile([1, P], FP)
            nc.tensor.transpose(tp1[:], tt[:, 1:2], idp)
            ts0 = pool.tile([1, P], FP)
            nc.scalar.copy(ts0[:], tp0[:])
            ts1 = pool.tile([1, P], FP)
            nc.scalar.copy(ts1[:], tp1[:])

            # staged rows for segmented exclusive scans over chunk index
            st0 = pool.tile([1, P], FP)
            nc.vector.tensor_copy(st0[0:1, 1:P], ts0[0:1, 0 : P - 1])
            nc.gpsimd.memset(st0[0:1, 0:P:CCH], 0.0)
            st1 = pool.tile([1, P], FP)
            nc.vector.tensor_copy(st1[0:1, 0 : P - 1], ts1[0:1, 1:P])
            nc.gpsimd.memset(st1[0:1, CCH - 1 : P : CCH], 0.0)

            # warm cross-engine deps of the STT-class scans on their engine
            scr3 = pool.tile([1, 1], FP)
            nc.vector.tensor_tensor(
                scr3[:], mr0[0:1, 0:1], st0[0:1, 0:1], Alu.max
            )
            scr4 = pool.tile([1, 1], FP)
            nc.vector.tensor_tensor(
                scr4[:], mr1[0:1, 0:1], st1[0:1, CCH - 1 : CCH], Alu.max
            )

            e20 = pool.tile([1, P], FP)
            nc.vector.tensor_tensor_scan(
                e20[:], mr0[:], st0[:], 0.0, Alu.mult, Alu.min
            )
            e21 = pool.tile([1, P], FP)
            nc.vector.tensor_tensor_scan(
                e21[0:1, ::-1], mr1[0:1, ::-1], st1[0:1, ::-1], 0.0,
                Alu.mult, Alu.min,
            )

            # back to per-partition columns
            ep0 = pps.tile([P, 1], FP)
            nc.tensor.transpose(ep0[:], e20[:], ones1[:])
            ep1 = pps.tile([P, 1], FP)
            nc.tensor.transpose(ep1[:], e21[:], ones1[:])

            # X = min(pf, E0) + (k_cell + C + 0.5) ; Y = min(wx, E1) + (C - k_cell - 0.5)
            xf = pool.tile([P, F], FP)
            nc.vector.scalar_tensor_tensor(
                xf[:], pf[:], ep0[:, 0:1], z1, Alu.min, Alu.add
            )
            yb = pool.tile([P, F], FP)
            nc.vector.scalar_tensor_tensor(
                yb[:], wx[:], ep1[:, 0:1], z2, Alu.min, Alu.add
            )
            md = pool.tile([P, F], FP)
            nc.vector.tensor_tensor(md[:], xf[:], yb[:], Alu.min)

            # out = exp(md * (-dx/sigma))
            ot = pool.tile([P, F], FP)
            nc.scalar.activation(
                ot[:], md[:], mybir.ActivationFunctionType.Exp, scale=dxs
            )
            nc.sync.dma_start(out=out[:], in_=ot[:])
    if compile:
        nc.compile()
    return nc


_NC_CACHE: bass.Bass | None = None


def _get_nc() -> bass.Bass:
    global _NC_CACHE
    if _NC_CACHE is None:
        _NC_CACHE = build_nc()
    return _NC_CACHE


def _host_inputs(state: np.ndarray, dx: float) -> list[dict[str, np.ndarray]]:
    s = np.ascontiguousarray(
        np.asarray(state, dtype=np.float32).reshape(B, NX)
    )
    # per-core [P, F+1]: partition (r, c) holds s[row, c*128-1 : c*128+128]
    # with a 2.0 pad for the non-existent s[row, -1] (kills interface k=0).
    padded = np.concatenate(
        [np.full((B, 1), 2.0, np.float32), s], axis=1
    )  # [B, NX+1]
    cidx = np.arange(CCH)[:, None] * F + np.arange(F + 1)[None, :]  # [16,129]

    p_idx = np.arange(P)
    kb = (p_idx % CCH).astype(np.float32)[:, None] * F  # [P,1]
    f = np.arange(F, dtype=np.float32)[None, :]         # [1,F]
    k = kb + f
    const = np.empty((P, W_IN), np.float32)
    const[:, O_X1 : O_X1 + F] = U_BIG - C_OFS - k
    const[:, O_X2 : O_X2 + F] = U_BIG - C_OFS + k
    const[:, O_Z1 : O_Z1 + F] = k + C_OFS + 0.5
    const[:, O_Z2 : O_Z2 + F] = C_OFS - k - 0.5
    const[:, O_ID : O_ID + P] = np.eye(P, dtype=np.float32)
    const[:, O_DXS] = -float(dx) / SIGMA

    j = np.arange(P)
    mrow = np.ones((2, P), np.float32)
    mrow[0, j % CCH == 0] = 0.0
    mrow[1, j % CCH == CCH - 1] = 0.0

    in_maps = []
    for core in range(N_CORES):
        rows = padded[core * R : (core + 1) * R]  # [R, NX+1]
        sp = rows[:, cidx.ravel()].reshape(R * CCH, F + 1)
        im = const.copy()
        im[:, O_SP : O_SP + F + 1] = sp
        in_maps.append({"inp": im, "mrow": mrow})
    return in_maps


def kernel(state: np.ndarray, dx) -> np.ndarray:
    dxv = float(np.asarray(dx).reshape(()))
    in_maps = _host_inputs(state, dxv)
    nc = _get_nc()
    res = run_bass_kernel_spmd(nc, in_maps, list(range(N_CORES))).results
    outs = [res[c]["out"].reshape(R, NX) for c in range(N_CORES)]
    full = np.concatenate(outs, axis=0).astype(np.float32)  # [B, NX]
    return full[:, None, :]


# revision 14
# speedup vs baseline: 1.5363x; 1.4536x over previous
"""Trainium2 Bass kernel for DifferentiableShockProximity.

Math: is_shock at interface k (k=1..nx-1) reduces to state[k] > state[k-1]
(the Greenshields Lax condition collapses to "density increases"). The
reference's O(nx^2) masked-distance min is a 1D nearest-shock distance
transform:

    min_dist(i) = dx * min( (i+0.5) + min_{k<=i}(u_k - k),
                           -(i+0.5) + min_{k>i}(u_k + k) )

with u_k = 0 at shocks, BIG elsewhere. Prefix/suffix mins run as hardware
tensor_tensor_scan ops along the free axis in a [128 partitions = (row,
chunk), 128 free = position-in-chunk] layout, with a small cross-chunk
combine through PE transposes + segmented scans over chunk totals.

All index arithmetic is exact in f32: integers (+0.5 offsets) below 2^24.
"0 means +inf" encoding: every real scan value is shifted by -2^21 so it
is negative; the multiplicative segment-reset of the cross-chunk scan
then yields 0, a natural +inf.

Data parallel over batch: 64 rows -> 8 cores x 8 rows. Host pads each
row-chunk with its left neighbor element so the shifted compare needs no
cross-partition traffic.
"""

import os
import sys

import numpy as np

for _p in (
    "/root/.axon_site/_ro/trn_rl_repo",
    "/opt/trn_rl_repo",
):
    if os.path.isdir(_p) and _p not in sys.path:
        sys.path.append(_p)

import concourse.bass as bass
import concourse.mybir as mybir
from concourse import bacc, masks
from concourse.bass_utils import run_bass_kernel_spmd
from concourse.tile import TileContext
from concourse.vector_clock import ScopedClock

N_CORES = 8
B, NX = 64, 2048
R = B // N_CORES  # rows per core
CCH = 16          # chunks per row
F = 128           # chunk length
P = R * CCH       # 128 partitions
C_OFS = float(2 ** 21)   # shift making every scan value negative
U_BIG = float(2 ** 20)   # "no shock" marker (index units)
SIGMA = 0.05
HF = F // 2

FP = mybir.dt.float32
Alu = mybir.AluOpType

# const tensor layout along free axis: X1 | Z1 | dxs
O_X1, O_Z1, O_DXS = 0, 128, 256
W_CST = 257


class _FastTileContext(TileContext):
    """TileContext with a cheap kernel tail.

    The stock exit emits drain + EVSEM-butterfly barrier + sem clear +
    second butterfly (~9 us on HW). This kernel is straight-line: once the
    final sync.drain has waited on the global vector clock, every
    semaphore increment has already happened, so a single sequencer-level
    (sem-only) barrier before the clear is enough, and nothing runs after
    the clear within this execution.
    """

    def _drain_and_barrier(self, tick_clock, wait_clock):
        drain_inst = self.nc.sync.drain()
        wait_clock.add_sem_waits(
            drain_inst.ins, ScopedClock({None: tick_clock.global_clock})
        )
        self.nc.all_engine_barrier(sem_only=True)
        assert self.sems is not None
        popped = self.nc._tile_sem_poison_stack.pop()
        assert popped is self._sem_poison
        self.nc.clear_and_free_semaphores(list(self.sems.allocated().values()))


def build_nc(compile: bool = True) -> bass.Bass:
    nc = bacc.Bacc(
        "TRN2", target_bir_lowering=False, debug=False, num_devices=N_CORES
    )
    spt = nc.declare_dram_parameter("spt", [P, F + 1], FP, isOutput=False)
    cst = nc.declare_dram_parameter("cst", [P, W_CST], FP, isOutput=False)
    out = nc.declare_dram_parameter("out", [P, F], FP, isOutput=True)

    with _FastTileContext(nc) as tc:
        with (
            tc.tile_pool(name="main", bufs=1) as pool,
            tc.tile_pool(name="ps", bufs=1, space="PSUM") as pps,
        ):
            # input state on the sync queue; constants in parallel on scalar's
            sp_t = pool.tile([P, F + 1], FP)
            nc.sync.dma_start(out=sp_t[:], in_=spt[:])
            ct = pool.tile([P, W_CST], FP)
            nc.scalar.dma_start(out=ct[:], in_=cst[:])
            x1 = ct[:, O_X1 : O_X1 + F]
            z1 = ct[:, O_Z1 : O_Z1 + F]
            dxs = ct[:, O_DXS : O_DXS + 1]

            # device-built constants (gpsimd is otherwise idle early)
            idp = pool.tile([P, P], FP)
            masks.make_identity(nc, idp[:])
            ones1 = pool.tile([1, 1], FP)
            nc.gpsimd.memset(ones1[:], 1.0)
            mr0 = pool.tile([1, P], FP)
            nc.gpsimd.memset(mr0[:], 1.0)
            nc.gpsimd.memset(mr0[0:1, 0:P:CCH], 0.0)
            mr1 = pool.tile([1, P], FP)
            nc.gpsimd.memset(mr1[:], 1.0)
            nc.gpsimd.memset(mr1[0:1, CCH - 1 : P : CCH], 0.0)

            # X2 = 2*(BIG-C) - X1, Z2 = 2*C - Z1 on the scalar engine
            x2 = pool.tile([P, F], FP)
            nc.scalar.activation(
                x2[:], x1, mybir.ActivationFunctionType.Copy,
                bias=2.0 * (U_BIG - C_OFS), scale=-1.0,
            )
            z2 = pool.tile([P, F], FP)
            nc.scalar.activation(
                z2[:], z1, mybir.ActivationFunctionType.Copy,
                bias=2.0 * C_OFS, scale=-1.0,
            )

            # mask: shock at interface k = chunk*128+f  <=>  s[k] > s[k-1]
            mask = pool.tile([P, F], FP)
            nc.vector.tensor_tensor(
                mask[:], sp_t[:, 1 : F + 1], sp_t[:, 0:F], Alu.is_gt
            )
            # vt = u - k - C = mask*(-BIG) + X1, wt = u + k - C = mask*(-BIG) + X2
            vt = pool.tile([P, F], FP)
            nc.vector.scalar_tensor_tensor(
                vt[:], mask[:], -U_BIG, x1, Alu.mult, Alu.add
            )
            wt = pool.tile([P, F], FP)
            nc.vector.scalar_tensor_tensor(
                wt[:], mask[:], -U_BIG, x2[:], Alu.mult, Alu.add
            )

            # chunk totals via direct reduces; feed the cross-chunk chain early
            tt = pool.tile([P, 2], FP)
            nc.vector.tensor_reduce(
                tt[:, 0:1], vt[:], mybir.AxisListType.X, Alu.min
            )
            nc.vector.tensor_reduce(
                tt[:, 1:2], wt[:], mybir.AxisListType.X, Alu.min
            )

            # PE transpose each totals column to a [1, P] row (PSUM)
            tp0 = pps.tile([1, P], FP)
            nc.tensor.transpose(tp0[:], tt[:, 0:1], idp[:])
            tp1 = pps.tile([1, P], FP)
            nc.tensor.transpose(tp1[:], tt[:, 1:2], idp[:])

            # staged rows for segmented exclusive scans over chunk index
            # (DVE reads PSUM directly)
            st0 = pool.tile([1, P], FP)
            nc.vector.tensor_copy(st0[0:1, 1:P], tp0[0:1, 0 : P - 1])
            nc.gpsimd.memset(st0[0:1, 0:P:CCH], 0.0)
            st1 = pool.tile([1, P], FP)
            nc.vector.tensor_copy(st1[0:1, 0 : P - 1], tp1[0:1, 1:P])
            nc.gpsimd.memset(st1[0:1, CCH - 1 : P : CCH], 0.0)

            e20 = pool.tile([1, P], FP)
            nc.vector.tensor_tensor_scan(
                e20[:], mr0[:], st0[:], 0.0, Alu.mult, Alu.min
            )
            e21 = pool.tile([1, P], FP)
            nc.vector.tensor_tensor_scan(
                e21[0:1, ::-1], mr1[0:1, ::-1], st1[0:1, ::-1], 0.0,
                Alu.mult, Alu.min,
            )

            # back to per-partition columns
            ep0 = pps.tile([P, 1], FP)
            nc.tensor.transpose(ep0[:], e20[:], ones1[:])
            ep1 = pps.tile([P, 1], FP)
            nc.tensor.transpose(ep1[:], e21[:], ones1[:])

            # chunk-local inclusive prefix-min of vt
            pf = pool.tile([P, F], FP)
            nc.vector.tensor_tensor_scan(
                pf[:], vt[:], vt[:], 0.0, Alu.min, Alu.min
            )
            # chunk-local exclusive suffix-min of wt (shift left, then
            # reversed scan); trailing slot = 0 (= +inf)
            wsh = pool.tile([P, F], FP)
            nc.vector.tensor_copy(wsh[:, 0 : F - 1], wt[:, 1:F])
            nc.gpsimd.memset(wsh[:, F - 1 : F], 0.0)
            wx = pool.tile([P, F], FP)
            nc.vector.tensor_tensor_scan(
                wx[:, ::-1], wsh[:, ::-1], wsh[:, ::-1], 0.0, Alu.min, Alu.min
            )

            # X = min(pf, E0) + (k_cell + C + 0.5) ; Y = min(wx, E1) + (C - k_cell - 0.5)
            xf = pool.tile([P, F], FP)
            nc.vector.scalar_tensor_tensor(
                xf[:], pf[:], ep0[:, 0:1], z1, Alu.min, Alu.add
            )
            yb = pool.tile([P, F], FP)
            nc.vector.scalar_tensor_tensor(
                yb[:], wx[:], ep1[:, 0:1], z2[:], Alu.min, Alu.add
            )

            # md = min(X, Y); out = exp(md * (-dx/sigma)); split halves so
            # the first output DMA overlaps the second half's exp
            md = pool.tile([P, F], FP)
            ot = pool.tile([P, F], FP)
            for h in range(2):
                sl = slice(h * HF, (h + 1) * HF)
                nc.vector.tensor_tensor(
                    md[:, sl], xf[:, sl], yb[:, sl], Alu.min
                )
                nc.scalar.activation(
                    ot[:, sl], md[:, sl],
                    mybir.ActivationFunctionType.Exp, scale=dxs,
                )
                nc.sync.dma_start(out=out[:, sl], in_=ot[:, sl])
    if compile:
        nc.compile()
    return nc


_NC_CACHE: bass.Bass | None = None


def _get_nc() -> bass.Bass:
    global _NC_CACHE
    if _NC_CACHE is None:
        _NC_CACHE = build_nc()
    return _NC_CACHE


def _host_inputs(state: np.ndarray, dx: float) -> list[dict[str, np.ndarray]]:
    s = np.ascontiguousarray(
        np.asarray(state, dtype=np.float32).reshape(B, NX)
    )
    # per-core [P, F+1]: partition (r, c) holds s[row, c*128-1 : c*128+128]
    # with a 2.0 pad for the non-existent s[row, -1] (kills interface k=0).
    padded = np.concatenate(
        [np.full((B, 1), 2.0, np.float32), s], axis=1
    )  # [B, NX+1]
    cidx = np.arange(CCH)[:, None] * F + np.arange(F + 1)[None, :]  # [16,129]

    p_idx = np.arange(P)
    kb = (p_idx % CCH).astype(np.float32)[:, None] * F  # [P,1]
    f = np.arange(F, dtype=np.float32)[None, :]         # [1,F]
    k = kb + f
    cst = np.empty((P, W_CST), np.float32)
    cst[:, O_X1 : O_X1 + F] = U_BIG - C_OFS - k
    cst[:, O_Z1 : O_Z1 + F] = k + C_OFS + 0.5
    cst[:, O_DXS] = -float(dx) / SIGMA

    in_maps = []
    for core in range(N_CORES):
        rows = padded[core * R : (core + 1) * R]  # [R, NX+1]
        sp = rows[:, cidx.ravel()].reshape(R * CCH, F + 1)
        in_maps.append({"spt": np.ascontiguousarray(sp), "cst": cst})
    return in_maps


def kernel(state: np.ndarray, dx) -> np.ndarray:
    dxv = float(np.asarray(dx).reshape(()))
    in_maps = _host_inputs(state, dxv)
    nc = _get_nc()
    res = run_bass_kernel_spmd(nc, in_maps, list(range(N_CORES))).results
    outs = [res[c]["out"].reshape(R, NX) for c in range(N_CORES)]
    full = np.concatenate(outs, axis=0).astype(np.float32)  # [B, NX]
    return full[:, None, :]
